# revision 1
# baseline (speedup 1.0000x reference)
"""GCN message-passing kernel for 8 TRN2 NeuronCores.

Reference computation (per (b, c) pair, all fp32):
    e1  = x @ W1^T + b1          [N, H]
    e2  = x @ W2^T + b2          [N, H]
    adj = relu(e1 @ e2^T)        [N, N]
    h   = adj @ x                [N, F]
    out = h @ W3^T + b3          [N, O]

Sharding: the 32 (b, c) pairs are split 4-per-core across 8 cores;
weights are replicated. Each core runs an identical Bass program fully
fused in SBUF/PSUM (the N x N adjacency never touches HBM).

Layout trick: partition p of SBUF holds rows [16p, 16p+16) of the pair
(a pure row permutation, applied consistently to both sides of every
contraction and undone by the output DMA), which makes every HBM
transfer contiguous 4KB per partition.
"""

import sys

for _p in ("/opt/trn_rl_repo",):
    if _p not in sys.path:
        sys.path.insert(0, _p)

import numpy as np

import concourse.bass as bass
import concourse.tile as tile
from concourse import bacc, mybir
from concourse.bass import ts
from concourse.masks import make_identity

B, C, N, F = 4, 8, 2048, 64
H = 64
O = 64
NCORES = 8
PAIRS = (B * C) // NCORES  # 4 (b,c) pairs per core
P = 128                    # SBUF partitions
TBLK = N // P              # 16 row-blocks per pair
CH = 512                   # moving-operand chunk (one PSUM bank of fp32)
NCH = N // CH              # 4 chunks per row
F32 = mybir.dt.float32
F32R = mybir.dt.float32r

AF = mybir.ActivationFunctionType

# Experiment flag: route the adjacency matmuls through transpose-mode
# (same algebra, 4x-faster FP32 stationary load path on cayman).
import os
ADJ_TMODE = os.environ.get("ADJ_TMODE", "0") == "1"


class _EngineAlternator:
    """Round-robin PSUM->SBUF copy/relu work across Scalar and Vector."""

    def __init__(self, nc):
        self.nc = nc
        self.i = 0

    def copy(self, out, in_):
        self.i += 1
        if self.i % 2:
            self.nc.scalar.copy(out, in_)
        else:
            self.nc.vector.tensor_copy(out, in_)

    def relu(self, out, in_):
        self.i += 1
        if self.i % 2:
            self.nc.scalar.activation(out, in_, AF.Relu)
        else:
            self.nc.vector.tensor_scalar_max(out, in_, 0.0)


def _emit(tc, x_d, w_d, b_d, out_d, reps=1, main_only=False):
    nc = tc.nc
    eng = _EngineAlternator(nc)

    import contextlib

    with contextlib.ExitStack() as ctx:
        consts = ctx.enter_context(tc.tile_pool(name="consts", bufs=1))
        xpool = ctx.enter_context(tc.tile_pool(name="xp", bufs=2))
        xtpool = ctx.enter_context(tc.tile_pool(name="xt", bufs=2))
        epool = ctx.enter_context(tc.tile_pool(name="ep", bufs=2))
        adjpool = ctx.enter_context(tc.tile_pool(name="adj", bufs=8))
        htpool = ctx.enter_context(tc.tile_pool(name="ht", bufs=2))
        opool = ctx.enter_context(tc.tile_pool(name="op", bufs=2))
        ps_adj = ctx.enter_context(tc.tile_pool(name="psa", bufs=4, space="PSUM"))
        ps_h = ctx.enter_context(tc.tile_pool(name="psh", bufs=4, space="PSUM"))

        ident = consts.tile([P, P], F32)
        make_identity(nc, ident[:])

        # Augmented transposed weights: wa[k] rows 0..F-1 = Wk^T, row F = bk,
        # so e = Wa^T @ [xT; ones] folds the bias into the matmul (K = F+1).
        # w1a/w2a feed fp32r matmuls, so every producer writes fp32r
        # (the BIR verifier requires fp32r operands to be pre-rounded).
        was = []
        for k in range(3):
            dt_k = F32 if k == 2 else F32R
            wraw = consts.tile([H, F], F32, tag="wraw")
            nc.sync.dma_start(wraw[:], w_d[k][:])
            brow = consts.tile([1, H], F32, tag="brow")
            nc.sync.dma_start(brow[:], b_d[k].unsqueeze(0))
            wa = consts.tile([F + 1, H], dt_k, tag=f"wa{k}")
            pw = ps_adj.tile([F, H], F32, tag="pa")
            nc.tensor.transpose(pw[:], wraw[:], ident[0:H, 0:H])
            nc.vector.tensor_copy(wa[0:F, :], pw[:])
            nc.vector.tensor_copy(wa[F : F + 1, :], brow[:])
            was.append(wa)
        w1a, w2a, w3a = was

        # Persistent xT tile (fp32r): Pool memset can't write fp32r, so the
        # ones row is seeded once from an fp32 staging row; rows 0..F-1 are
        # overwritten per pair.
        ones_row = consts.tile([1, N], F32)
        nc.gpsimd.memset(ones_row[:], 1.0)
        xta = consts.tile([F + 1, N], F32R)
        nc.vector.tensor_copy(xta[F : F + 1, :], ones_row[:])

        def prep(p):
            """Pair prologue: x load, fp32r round, xT transposes, e1T/e2T.
            Emitted mid-way through the previous pair's main loop so the PE
            queue never drains at a pair boundary."""
            x_sb = xpool.tile([P, TBLK, F], F32, tag="x_sb", name=f"x_sb{p}")
            nc.sync.dma_start(
                x_sb[:], x_d[p].rearrange("(q t) f -> q t f", q=P)
            )
            # fp32r-rounded copy of x for the hT-accumulation stationaries
            x_sbr = xpool.tile([P, TBLK * F], F32R, tag="x_sbr", name=f"x_sbr{p}")
            eng.copy(x_sbr[:], x_sb[:].rearrange("q t f -> q (t f)"))

            # xT (augmented with ones row): column t*128+q <-> row 16q+t
            for c in range(NCH):
                pt = ps_adj.tile([F, CH], F32, tag="pa", name=f"pt{p}_{c}")
                for j in range(CH // P):
                    t = c * (CH // P) + j
                    nc.tensor.transpose(pt[:, ts(j, P)], x_sb[:, t, :], ident[:])
                eng.copy(xta[0:F, ts(c, CH)], pt[:])

            # e1T / e2T = Wa^T @ xTa   (biases included via K=65)
            ets = []
            for wa, tag in ((w1a, "e1t"), (w2a, "e2t")):
                et = epool.tile([H, N], F32R, tag=tag, name=f"{tag}{p}")
                for c in range(NCH):
                    pe_ = ps_adj.tile([H, CH], F32, tag="pa", name=f"pe{p}_{c}")
                    nc.tensor.matmul(
                        pe_[:], wa[:], xta[:, ts(c, CH)], start=True, stop=True
                    )
                    eng.copy(et[:, ts(c, CH)], pe_[:])
                ets.append(et)
            return x_sb, x_sbr, ets[0], ets[1]

        def main(p, st, tail_emit):
            """Main fused loop for pair p. Interleaves the deferred tail of
            pair p-1 (early) and the prologue of pair p+1 (late)."""
            x_sb, x_sbr, e1t, e2t = st
            ph = [
                ps_h.tile([F, CH], F32, tag="ph", name=f"ph{p}_{c}")
                for c in range(NCH)
            ]
            next_st = None
            def emit_adj(mb):
                pas = []
                for c in range(NCH):
                    if ADJ_TMODE:
                        pa = ps_adj.tile([P, CH], F32R, tag="pa",
                                         name=f"pa{p}_{mb}_{c}")
                        nc.tensor.matmul(
                            pa[:], e2t[:, ts(mb, P)], e1t[:, ts(c, CH)],
                            start=True, stop=True, is_transpose=True,
                        )
                    else:
                        pa = ps_adj.tile([P, CH], F32, tag="pa",
                                         name=f"pa{p}_{mb}_{c}")
                        nc.tensor.matmul(
                            pa[:], e2t[:, ts(mb, P)], e1t[:, ts(c, CH)],
                            start=True, stop=True,
                        )
                    pas.append(pa)
                return pas

            # one-block lookahead: adj(mb+1) is emitted (and thus scheduled
            # on the PE) BEFORE hT(mb), which has to wait for relu(mb) — so
            # the PE chews adjacency blocks while the relu engines drain.
            pas = emit_adj(0)
            for mb in range(TBLK):
                asbs = []
                for c in range(NCH):
                    asb = adjpool.tile([P, CH], F32R, tag="asb",
                                       name=f"asb{p}_{mb}_{c}")
                    eng.relu(asb[:], pas[c][:])
                    asbs.append(asb)
                if mb + 1 < TBLK:
                    pas = emit_adj(mb + 1)
                for c in range(NCH):
                    nc.tensor.matmul(
                        ph[c][:], x_sbr[:, ts(mb, F)], asbs[c][:],
                        start=(mb == 0), stop=(mb == TBLK - 1),
                    )
                if mb == 3 and tail_emit is not None:
                    tail_emit()
                    tail_emit = None
                if mb == 8 and p + 1 < PAIRS:
                    next_st = prep(p + 1)

            # hT -> SBUF right away (frees the 4 ph PSUM banks; DVE/ACT work
            # that overlaps the next pair's PE stream)
            hta = htpool.tile([F + 1, N], F32, tag="hta", name=f"hta{p}")
            nc.gpsimd.memset(hta[F : F + 1, :], 1.0)
            for c in range(NCH):
                eng.copy(hta[0:F, ts(c, CH)], ph[c][:])

            def tail():
                # out = hTa^T @ W3a  (b3 via K=65), packed 8 blocks/bank
                out_sb = opool.tile([P, TBLK * O], F32, tag="out_sb",
                                    name=f"out_sb{p}")
                for g in range(2):
                    po = ps_adj.tile([P, CH], F32, tag="pa", name=f"po{p}_{g}")
                    for j in range(TBLK // 2):
                        nb = g * (TBLK // 2) + j
                        nc.tensor.matmul(
                            po[:, ts(j, O)], hta[:, ts(nb, P)], w3a[:],
                            start=True, stop=True,
                        )
                    eng.copy(out_sb[:, ts(g, CH)], po[:])
                nc.sync.dma_start(
                    out_d[p].rearrange("(q t) f -> q (t f)", q=P), out_sb[:]
                )

            return next_st, tail

        def body():
            if main_only:
                st = prep(0)
                for p in range(PAIRS):
                    ph = [
                        ps_h.tile([F, CH], F32, tag="ph", name=f"mph{p}_{c}")
                        for c in range(NCH)
                    ]
                    x_sb, x_sbr, e1t, e2t = st
                    for mb in range(TBLK):
                        for c in range(NCH):
                            pa = ps_adj.tile([P, CH], F32, tag="pa",
                                             name=f"mpa{p}_{mb}_{c}")
                            nc.tensor.matmul(
                                pa[:], e2t[:, ts(mb, P)], e1t[:, ts(c, CH)],
                                start=True, stop=True,
                            )
                            asb = adjpool.tile([P, CH], F32R, tag="asb",
                                               name=f"masb{p}_{mb}_{c}")
                            eng.relu(asb[:], pa[:])
                            nc.tensor.matmul(
                                ph[c][:], x_sbr[:, ts(mb, F)], asb[:],
                                start=(mb == 0), stop=(mb == TBLK - 1),
                            )
                    hta = htpool.tile([F + 1, N], F32, tag="hta",
                                      name=f"mhta{p}")
                    for c in range(NCH):
                        eng.copy(hta[0:F, ts(c, CH)], ph[c][:])
                    nc.sync.dma_start(
                        out_d[p].rearrange("(q t) f -> q (t f)", q=P)[0:F, :],
                        hta[0:F, 0 : TBLK * F],
                    )
                return
            st = prep(0)
            tail = None
            for p in range(PAIRS):
                st, tail = main(p, st, tail)
            tail()

        if reps == 1:
            body()
        else:
            with tc.For_i(0, reps, 1):
                body()


def build_program(reps=1, main_only=False):
    nc = bacc.Bacc("TRN2", target_bir_lowering=False, debug=False)
    x_d = nc.dram_tensor("x", [PAIRS, N, F], F32, kind="ExternalInput").ap()
    w_d = [
        nc.dram_tensor(f"w{k}", [H, F], F32, kind="ExternalInput").ap()
        for k in (1, 2, 3)
    ]
    b_d = [
        nc.dram_tensor(f"b{k}", [H], F32, kind="ExternalInput").ap()
        for k in (1, 2, 3)
    ]
    out_d = nc.dram_tensor("out", [PAIRS, N, O], F32, kind="ExternalOutput").ap()
    with tile.TileContext(nc) as tc:
        _emit(tc, x_d, w_d, b_d, out_d, reps=reps, main_only=main_only)
    nc.compile()
    return nc


def make_in_maps(x, W1, b1, W2, b2, W3, b3):
    xs = np.ascontiguousarray(np.asarray(x, np.float32).reshape(B * C, N, F))
    const = {
        "w1": np.ascontiguousarray(np.asarray(W1, np.float32)),
        "w2": np.ascontiguousarray(np.asarray(W2, np.float32)),
        "w3": np.ascontiguousarray(np.asarray(W3, np.float32)),
        "b1": np.ascontiguousarray(np.asarray(b1, np.float32)),
        "b2": np.ascontiguousarray(np.asarray(b2, np.float32)),
        "b3": np.ascontiguousarray(np.asarray(b3, np.float32)),
    }
    return [
        {"x": np.ascontiguousarray(xs[i * PAIRS : (i + 1) * PAIRS]), **const}
        for i in range(NCORES)
    ]


_NC_CACHE = {}


def kernel(x, W1, b1, W2, b2, W3, b3):
    from concourse.bass_utils import run_bass_kernel_spmd

    if "nc" not in _NC_CACHE:
        _NC_CACHE["nc"] = build_program()
    nc = _NC_CACHE["nc"]
    in_maps = make_in_maps(x, W1, b1, W2, b2, W3, b3)
    res = run_bass_kernel_spmd(nc, in_maps, list(range(NCORES))).results
    out = np.concatenate([res[i]["out"] for i in range(NCORES)], axis=0)
    return out.reshape(B, C, N, O)



# revision 2
# speedup vs baseline: 1.5018x; 1.5018x over previous
"""GCN message-passing kernel for 8 TRN2 NeuronCores.

Reference computation (per (b, c) pair, fp32 reference):
    e1  = x @ W1^T + b1          [N, H]
    e2  = x @ W2^T + b2          [N, H]
    adj = relu(e1 @ e2^T)        [N, N]
    h   = adj @ x                [N, F]
    out = h @ W3^T + b3          [N, O]

Sharding: the 32 (b, c) pairs are split 4-per-core across 8 cores;
weights are replicated. Each core runs an identical Bass program fully
fused in SBUF/PSUM (the N x N adjacency never touches HBM).

All matmul operands are bf16 (PSUM accumulation stays fp32) — on HW the
PE streams bf16 at 1 cycle/row vs ~2 for fp32r, and the 2e-2 rel-err
gate leaves ample room (bf16 quantization contributes ~3e-3).

Layout trick: partition p of SBUF holds rows [16p, 16p+16) of the pair
(a pure row permutation, applied consistently to both sides of every
contraction and undone by the output DMA), which makes every HBM
transfer contiguous 4KB per partition.
"""

import sys

for _p in ("/opt/trn_rl_repo",):
    if _p not in sys.path:
        sys.path.insert(0, _p)

import numpy as np

import concourse.bass as bass
import concourse.tile as tile
from concourse import bacc, mybir
from concourse.bass import ts
from concourse.masks import make_identity

B, C, N, F = 4, 8, 2048, 64
H = 64
O = 64
NCORES = 8
PAIRS = (B * C) // NCORES  # 4 (b,c) pairs per core
P = 128                    # SBUF partitions
TBLK = N // P              # 16 row-blocks per pair
CH = 512                   # moving-operand chunk (one PSUM bank of fp32)
NCH = N // CH              # 4 chunks per row
F32 = mybir.dt.float32
BF16 = mybir.dt.bfloat16

AF = mybir.ActivationFunctionType


class _EngineAlternator:
    """Round-robin PSUM->SBUF copy/relu work across Scalar and Vector."""

    def __init__(self, nc):
        self.nc = nc
        self.i = 0

    def copy(self, out, in_):
        self.i += 1
        if self.i % 2:
            self.nc.scalar.copy(out, in_)
        else:
            self.nc.vector.tensor_copy(out, in_)

    def relu(self, out, in_):
        self.i += 1
        if self.i % 2:
            self.nc.scalar.activation(out, in_, AF.Relu)
        else:
            self.nc.vector.tensor_scalar_max(out, in_, 0.0)


def _emit(tc, x_d, w_d, b_d, out_d, reps=1):
    nc = tc.nc
    eng = _EngineAlternator(nc)

    import contextlib

    with contextlib.ExitStack() as ctx:
        consts = ctx.enter_context(tc.tile_pool(name="consts", bufs=1))
        xpool = ctx.enter_context(tc.tile_pool(name="xp", bufs=2))
        epool = ctx.enter_context(tc.tile_pool(name="ep", bufs=2))
        adjpool = ctx.enter_context(tc.tile_pool(name="adj", bufs=8))
        htpool = ctx.enter_context(tc.tile_pool(name="ht", bufs=2))
        opool = ctx.enter_context(tc.tile_pool(name="op", bufs=2))
        ps_adj = ctx.enter_context(tc.tile_pool(name="psa", bufs=4, space="PSUM"))
        ps_h = ctx.enter_context(tc.tile_pool(name="psh", bufs=4, space="PSUM"))

        ident = consts.tile([P, P], F32)
        make_identity(nc, ident[:])
        identb = consts.tile([P, P], BF16)
        make_identity(nc, identb[:])

        # Augmented transposed weights (bf16): wa[k] rows 0..F-1 = Wk^T,
        # row F = bk, so e = Wa^T @ [xT; ones] folds the bias into the
        # matmul (K = F+1).
        was = []
        for k in range(3):
            wraw = consts.tile([H, F], F32, tag="wraw")
            nc.sync.dma_start(wraw[:], w_d[k][:])
            brow = consts.tile([1, H], F32, tag="brow")
            nc.sync.dma_start(brow[:], b_d[k].unsqueeze(0))
            wa = consts.tile([F + 1, H], BF16, tag=f"wa{k}")
            pw = ps_adj.tile([F, H], F32, tag="pa")
            nc.tensor.transpose(pw[:], wraw[:], ident[0:H, 0:H])
            nc.vector.tensor_copy(wa[0:F, :], pw[:])
            nc.vector.tensor_copy(wa[F : F + 1, :], brow[:])
            was.append(wa)
        w1a, w2a, w3a = was

        # Persistent augmented-xT tile (bf16): ones row seeded once, rows
        # 0..F-1 overwritten per pair.
        xta = consts.tile([F + 1, N], BF16)
        nc.gpsimd.memset(xta[F : F + 1, :], 1.0)

        def prep(p):
            """Pair prologue: x load, bf16 round, xT transposes, e1T/e2T.
            Emitted mid-way through the previous pair's main loop so the PE
            queue never drains at a pair boundary."""
            x_sb = xpool.tile([P, TBLK, F], F32, tag="x_sb", name=f"x_sb{p}")
            nc.sync.dma_start(
                x_sb[:], x_d[p].rearrange("(q t) f -> q t f", q=P)
            )
            # bf16 copy of x: transpose source AND the hT-accumulation
            # stationaries.
            x_bf = xpool.tile([P, TBLK, F], BF16, tag="x_bf", name=f"x_bf{p}")
            eng.copy(x_bf[:], x_sb[:])

            # xT (augmented with ones row): column t*128+q <-> row 16q+t
            for c in range(NCH):
                pt = ps_adj.tile([F, CH], BF16, tag="pa", name=f"pt{p}_{c}")
                for j in range(CH // P):
                    t = c * (CH // P) + j
                    nc.tensor.transpose(pt[:, ts(j, P)], x_bf[:, t, :], identb[:])
                eng.copy(xta[0:F, ts(c, CH)], pt[:])

            # e1T / e2T = Wa^T @ xTa   (biases included via K=65)
            ets = []
            for wa, tag in ((w1a, "e1t"), (w2a, "e2t")):
                et = epool.tile([H, N], BF16, tag=tag, name=f"{tag}{p}")
                for c in range(NCH):
                    pe_ = ps_adj.tile([H, CH], F32, tag="pa", name=f"pe{p}_{c}")
                    nc.tensor.matmul(
                        pe_[:], wa[:], xta[:, ts(c, CH)], start=True, stop=True
                    )
                    eng.copy(et[:, ts(c, CH)], pe_[:])
                ets.append(et)
            return x_bf, ets[0], ets[1]

        def main(p, st, tail_emit):
            """Main fused loop for pair p. Interleaves the deferred tail of
            pair p-1 (early) and the prologue of pair p+1 (late)."""
            x_bf, e1t, e2t = st
            ph = [
                ps_h.tile([F, CH], F32, tag="ph", name=f"ph{p}_{c}")
                for c in range(NCH)
            ]
            next_st = None

            def emit_adj(mb):
                pas = []
                for c in range(NCH):
                    pa = ps_adj.tile([P, CH], F32, tag="pa",
                                     name=f"pa{p}_{mb}_{c}")
                    nc.tensor.matmul(
                        pa[:], e2t[:, ts(mb, P)], e1t[:, ts(c, CH)],
                        start=True, stop=True,
                    )
                    pas.append(pa)
                return pas

            # one-block lookahead: adj(mb+1) is emitted (and thus scheduled
            # on the PE) BEFORE hT(mb), which has to wait for relu(mb) — so
            # the PE chews adjacency blocks while the relu engines drain.
            pas = emit_adj(0)
            for mb in range(TBLK):
                asbs = []
                for c in range(NCH):
                    asb = adjpool.tile([P, CH], BF16, tag="asb",
                                       name=f"asb{p}_{mb}_{c}")
                    eng.relu(asb[:], pas[c][:])
                    asbs.append(asb)
                if mb + 1 < TBLK:
                    pas = emit_adj(mb + 1)
                for c in range(NCH):
                    nc.tensor.matmul(
                        ph[c][:], x_bf[:, mb, :], asbs[c][:],
                        start=(mb == 0), stop=(mb == TBLK - 1),
                    )
                if mb == 3 and tail_emit is not None:
                    tail_emit()
                    tail_emit = None
                if mb == 8 and p + 1 < PAIRS:
                    next_st = prep(p + 1)

            # hT -> SBUF right away (frees the 4 ph PSUM banks; DVE/ACT work
            # that overlaps the next pair's PE stream)
            hta = htpool.tile([F + 1, N], BF16, tag="hta", name=f"hta{p}")
            nc.gpsimd.memset(hta[F : F + 1, :], 1.0)
            for c in range(NCH):
                eng.copy(hta[0:F, ts(c, CH)], ph[c][:])

            def tail():
                # out = hTa^T @ W3a  (b3 via K=65), packed 8 blocks/bank
                out_sb = opool.tile([P, TBLK * O], F32, tag="out_sb",
                                    name=f"out_sb{p}")
                for g in range(2):
                    po = ps_adj.tile([P, CH], F32, tag="pa", name=f"po{p}_{g}")
                    for j in range(TBLK // 2):
                        nb = g * (TBLK // 2) + j
                        nc.tensor.matmul(
                            po[:, ts(j, O)], hta[:, ts(nb, P)], w3a[:],
                            start=True, stop=True,
                        )
                    eng.copy(out_sb[:, ts(g, CH)], po[:])
                nc.sync.dma_start(
                    out_d[p].rearrange("(q t) f -> q (t f)", q=P), out_sb[:]
                )

            return next_st, tail

        def body():
            st = prep(0)
            tail = None
            for p in range(PAIRS):
                st, tail = main(p, st, tail)
            tail()

        if reps == 1:
            body()
        else:
            with tc.For_i(0, reps, 1):
                body()


def build_program(reps=1):
    nc = bacc.Bacc("TRN2", target_bir_lowering=False, debug=False)
    x_d = nc.dram_tensor("x", [PAIRS, N, F], F32, kind="ExternalInput").ap()
    w_d = [
        nc.dram_tensor(f"w{k}", [H, F], F32, kind="ExternalInput").ap()
        for k in (1, 2, 3)
    ]
    b_d = [
        nc.dram_tensor(f"b{k}", [H], F32, kind="ExternalInput").ap()
        for k in (1, 2, 3)
    ]
    out_d = nc.dram_tensor("out", [PAIRS, N, O], F32, kind="ExternalOutput").ap()
    with tile.TileContext(nc) as tc:
        _emit(tc, x_d, w_d, b_d, out_d, reps=reps)
    nc.compile()
    return nc


def make_in_maps(x, W1, b1, W2, b2, W3, b3):
    xs = np.ascontiguousarray(np.asarray(x, np.float32).reshape(B * C, N, F))
    const = {
        "w1": np.ascontiguousarray(np.asarray(W1, np.float32)),
        "w2": np.ascontiguousarray(np.asarray(W2, np.float32)),
        "w3": np.ascontiguousarray(np.asarray(W3, np.float32)),
        "b1": np.ascontiguousarray(np.asarray(b1, np.float32)),
        "b2": np.ascontiguousarray(np.asarray(b2, np.float32)),
        "b3": np.ascontiguousarray(np.asarray(b3, np.float32)),
    }
    return [
        {"x": np.ascontiguousarray(xs[i * PAIRS : (i + 1) * PAIRS]), **const}
        for i in range(NCORES)
    ]


_NC_CACHE = {}


def kernel(x, W1, b1, W2, b2, W3, b3):
    from concourse.bass_utils import run_bass_kernel_spmd

    if "nc" not in _NC_CACHE:
        _NC_CACHE["nc"] = build_program()
    nc = _NC_CACHE["nc"]
    in_maps = make_in_maps(x, W1, b1, W2, b2, W3, b3)
    res = run_bass_kernel_spmd(nc, in_maps, list(range(NCORES))).results
    out = np.concatenate([res[i]["out"] for i in range(NCORES)], axis=0)
    return out.reshape(B, C, N, O)


# revision 8
# speedup vs baseline: 1.5805x; 1.0524x over previous
"""GCN message-passing kernel for 8 TRN2 NeuronCores.

Reference computation (per (b, c) pair, fp32 reference):
    e1  = x @ W1^T + b1          [N, H]
    e2  = x @ W2^T + b2          [N, H]
    adj = relu(e1 @ e2^T)        [N, N]
    h   = adj @ x                [N, F]
    out = h @ W3^T + b3          [N, O]

Sharding: the 32 (b, c) pairs are split 4-per-core across 8 cores;
weights are replicated. Each core runs an identical Bass program fully
fused in SBUF/PSUM (the N x N adjacency never touches HBM).

All matmul operands are bf16 (PSUM accumulation stays fp32); the 2e-2
rel-err gate leaves ample room (bf16 quantization contributes ~3e-3).

Parity layout: every K=64 contraction is packed two-per-PE via
tile_position row tiles (0,*) / (64,*), which the PE executes
concurrently (measured 2x). Row-block t of a pair lives on partition
half t%2: the [128,128] x-transposes emit (f@t-even ; f@t-odd) stacked,
and e1/e2/h^T inherit the layout, so adjacency, the e-matmuls and the
output projection all pair up. biases ride along on the mandatory
PSUM->SBUF copies (per-partition bias on ACT, tensor-tensor add on DVE)
instead of K=65 augmentation, keeping contractions pairable at K=64.

Layout trick: partition p of SBUF holds rows [16p, 16p+16) of the pair
(a pure row permutation, applied consistently to both sides of every
contraction and undone by the output DMA), which makes every HBM
transfer contiguous 4KB per partition.
"""

import sys

for _p in ("/opt/trn_rl_repo",):
    if _p not in sys.path:
        sys.path.insert(0, _p)

import numpy as np

import concourse.bass as bass
import concourse.tile as tile
from concourse import bacc, mybir
from concourse.bass import ts
from concourse.masks import make_identity

B, C, N, F = 4, 8, 2048, 64
H = 64
O = 64
NCORES = 8
PAIRS = (B * C) // NCORES  # 4 (b,c) pairs per core
P = 128                    # SBUF partitions
HP = 64                    # half-partition (PE row/col tile)
TBLK = N // P              # 16 row-blocks per pair
NPAIR_T = TBLK // 2        # 8 block-pairs (even t, odd t)
CH = 512                   # moving-operand chunk (one PSUM bank of fp32)
NH = N // 2                # 1024 columns per parity
NCH2 = NH // CH            # 2 chunks per parity
F32 = mybir.dt.float32
BF16 = mybir.dt.bfloat16

AF = mybir.ActivationFunctionType
ID = AF.Identity


class _Rotor:
    """Alternate PSUM->SBUF relu/copy work across DVE and ACT.
    (GPSIMD cannot access PSUM on TRN2, so Pool only gets SBUF-only work.)"""

    def __init__(self, nc):
        self.nc = nc
        self.i = 0

    def relu(self, out, in_):
        e = "da"[self.i % 2]
        self.i += 1
        if e == "d":
            self.nc.vector.tensor_scalar_max(out, in_, 0.0)
        else:
            self.nc.scalar.activation(out, in_, AF.Relu)

    def copy(self, out, in_):
        e = "da"[self.i % 2]
        self.i += 1
        if e == "d":
            self.nc.vector.tensor_copy(out, in_)
        else:
            self.nc.scalar.copy(out, in_)


def _emit(tc, x_d, w_d, b_d, out_d, reps=1):
    nc = tc.nc
    rot = _Rotor(nc)

    import contextlib

    with contextlib.ExitStack() as ctx:
        consts = ctx.enter_context(tc.tile_pool(name="consts", bufs=1))
        xpool = ctx.enter_context(tc.tile_pool(name="xp", bufs=2))
        xtpool = ctx.enter_context(tc.tile_pool(name="xt", bufs=2))
        epool = ctx.enter_context(tc.tile_pool(name="ep", bufs=2))
        adjpool = ctx.enter_context(tc.tile_pool(name="adj", bufs=8))
        htpool = ctx.enter_context(tc.tile_pool(name="ht", bufs=2))
        opool = ctx.enter_context(tc.tile_pool(name="op", bufs=2))
        ps_adj = ctx.enter_context(tc.tile_pool(name="psa", bufs=6, space="PSUM"))
        ps_h = ctx.enter_context(tc.tile_pool(name="psh", bufs=2, space="PSUM"))

        ident = consts.tile([P, P], F32)
        make_identity(nc, ident[:])
        identb = consts.tile([P, P], BF16)
        make_identity(nc, identb[:])

        # Weights W1/W2/W3 transposed to [f, h] bf16, duplicated on both
        # partition halves (for row-tile pairing); per-partition bias cols.
        wstats, bcols = [], []
        for k in range(3):
            wraw = consts.tile([H, F], F32, tag="wraw")
            nc.sync.dma_start(wraw[:], w_d[k][:])
            wst = consts.tile([P, H], BF16, tag=f"wst{k}")
            pw = ps_adj.tile([F, H], F32, tag="pa")
            nc.tensor.transpose(pw[:], wraw[:], ident[0:H, 0:H])
            nc.vector.tensor_copy(wst[0:HP, :], pw[:])
            nc.sync.dma_start(wst[HP:P, :], wst[0:HP, :])
            wstats.append(wst)
            if k < 2:
                bcol = consts.tile([P, 1], F32, tag=f"bc{k}")
                nc.sync.dma_start(bcol[0:HP, :], b_d[k].unsqueeze(1))
                nc.sync.dma_start(bcol[HP:P, :], b_d[k].unsqueeze(1))
                bcols.append(bcol)
        w1st, w2st, w3st = wstats
        b1c, b2c = bcols

        # b3 replicated [128, 512] (cols = 8 blocks x 64 o) for the fused
        # bias-add on the projection's PSUM->SBUF copy.
        b3row = consts.tile([1, O], F32)
        nc.sync.dma_start(b3row[:], b_d[2].unsqueeze(0))
        b3x8 = consts.tile([1, CH], F32)
        for j in range(CH // O):
            nc.vector.tensor_copy(b3x8[:, ts(j, O)], b3row[:])
        ones1 = consts.tile([1, P], F32)
        nc.gpsimd.memset(ones1[:], 1.0)
        b3rep = consts.tile([P, CH], F32)
        pb3 = ps_adj.tile([P, CH], F32, tag="pa")
        nc.tensor.matmul(pb3[:], ones1[:], b3x8[:], start=True, stop=True)
        nc.vector.tensor_copy(b3rep[:], pb3[:])

        def prep(p):
            """Pair prologue: x load, bf16 round, parity transposes,
            e1/e2 (+ swapped-half copy of e1)."""
            x_sb = xpool.tile([P, TBLK, F], F32, tag="x_sb", name=f"x_sb{p}")
            nc.sync.dma_start(
                x_sb[:], x_d[p].rearrange("(q t) f -> q t f", q=P)
            )
            # SBUF->SBUF, so Pool can own it (frees DVE/ACT for PSUM work)
            x_bf = xpool.tile([P, TBLK, F], BF16, tag="x_bf", name=f"x_bf{p}")
            nc.gpsimd.tensor_copy(x_bf[:], x_sb[:])

            # xT in parity layout: column (c', q) holds f@t=2c' on rows
            # 0..63 and f@t=2c'+1 on rows 64..127.
            xta = xtpool.tile([P, NH], BF16, tag="xta", name=f"xta{p}")
            for cp in range(NPAIR_T):
                pt = ps_adj.tile([P, P], BF16, tag="pa", name=f"pt{p}_{cp}")
                nc.tensor.transpose(
                    pt[:], x_bf[:, 2 * cp : 2 * cp + 2, :], identb[:]
                )
                rot.copy(xta[:, ts(cp, P)], pt[:])

            # e1/e2 in parity layout, paired diagonal tiles (0,0)/(64,64);
            # bias added on the ACT PSUM->SBUF copy (per-partition bias AP).
            e1t = epool.tile([P, NH], BF16, tag="e1t", name=f"e1t{p}")
            e2t = epool.tile([P, NH], BF16, tag="e2t", name=f"e2t{p}")
            for wst, bc, et in ((w1st, b1c, e1t), (w2st, b2c, e2t)):
                for g in range(NCH2):
                    pe2 = ps_adj.tile([P, CH], F32, tag="pa",
                                      name=f"pe{p}_{g}")
                    nc.tensor.matmul(
                        pe2[0:HP, :], wst[0:HP, :], xta[0:HP, ts(g, CH)],
                        start=True, stop=True,
                    )
                    nc.tensor.matmul(
                        pe2[HP:P, :], wst[HP:P, :], xta[HP:P, ts(g, CH)],
                        start=True, stop=True,
                    )
                    nc.scalar.activation(et[0:HP, ts(g, CH)], pe2[0:HP, :],
                                         ID, bias=bc[0:HP, :])
                    nc.scalar.activation(et[HP:P, ts(g, CH)], pe2[HP:P, :],
                                         ID, bias=bc[HP:P, :])
            # swapped-half copy of e1 so any (row-parity, col-parity) adj
            # instruction finds its moving operand on the right partitions.
            e1s = epool.tile([P, NH], BF16, tag="e1s", name=f"e1s{p}")
            nc.sync.dma_start(e1s[0:HP, :], e1t[HP:P, :])
            nc.sync.dma_start(e1s[HP:P, :], e1t[0:HP, :])
            return x_bf, e1t, e1s, e2t

        CHUNKS = [(pc, g) for g in range(NCH2) for pc in range(2)]

        def main(p, st, tail_emit):
            """Main fused loop for pair p, over 8 block-pairs (even t /
            odd t on PE row-tiles 0 / 64, concurrent)."""
            x_bf, e1t, e1s, e2t = st
            ph = [
                ps_h.tile([P, CH], F32, tag="ph", name=f"ph{p}_{g}")
                for g in range(NCH2)
            ]
            next_st = None
            for u in range(NPAIR_T):
                asbs = {}
                for ci, (pc, g) in enumerate(CHUNKS):
                    pas = []
                    for pt_ in range(2):  # t = 2u + pt_, row-tile 64*pt_
                        mv = e1t if pc == pt_ else e1s
                        pa = ps_adj.tile([P, CH], F32, tag="pa",
                                         name=f"pa{p}_{u}_{pc}_{g}_{pt_}")
                        nc.tensor.matmul(
                            pa[:],
                            e2t[HP * pt_ : HP * pt_ + HP, ts(u, P)],
                            mv[HP * pt_ : HP * pt_ + HP, ts(g, CH)],
                            start=True, stop=True,
                        )
                        pas.append(pa)
                    for pt_ in range(2):
                        asb = adjpool.tile([P, CH], BF16, tag="asb",
                                           name=f"asb{p}_{u}_{pc}_{g}_{pt_}")
                        rot.relu(asb[:], pas[pt_][:])
                        asbs[(pc, g, pt_)] = asb
                for ci, (pc, g) in enumerate(CHUNKS):
                    for pt_ in range(2):
                        # skip_group_check: the interp's PSUM group tracker
                        # is partition-blind; the two half-bank groups are
                        # on disjoint partitions (HW zeroes per element).
                        nc.tensor.matmul(
                            ph[g][HP * pc : HP * pc + HP, :],
                            x_bf[:, 2 * u + pt_, :],
                            asbs[(pc, g, pt_)][:],
                            start=(u == 0 and pt_ == 0),
                            stop=(u == NPAIR_T - 1 and pt_ == 1),
                            skip_group_check=True,
                        )
                if u == 1 and tail_emit is not None:
                    tail_emit()
                    tail_emit = None
                if u == 4 and p + 1 < PAIRS:
                    next_st = prep(p + 1)

            # hT -> SBUF (parity layout [128, NH]): frees the ph banks.
            hta = htpool.tile([P, NH], BF16, tag="hta", name=f"hta{p}")
            for g in range(NCH2):
                rot.copy(hta[0:HP, ts(g, CH)], ph[g][0:HP, :])
                rot.copy(hta[HP:P, ts(g, CH)], ph[g][HP:P, :])

            def tail():
                # out = h @ W3^T + b3: per t-block, stationary
                # hta[parity-half, c'-block] (paired row tiles), moving
                # W3^T; + b3 fused into the PSUM->SBUF tensor-tensor add.
                # out_sb free layout [cp, par, o] == [(t) o] row-major
                out_sb = opool.tile([P, NPAIR_T, 2, O], F32, tag="out_sb",
                                    name=f"out_sb{p}")
                poe = ps_adj.tile([P, CH], F32, tag="pa", name=f"poe{p}")
                poo = ps_adj.tile([P, CH], F32, tag="pa", name=f"poo{p}")
                for cp in range(NPAIR_T):
                    nc.tensor.matmul(
                        poe[:, ts(cp, O)], hta[0:HP, ts(cp, P)],
                        w3st[0:HP, :], start=True, stop=True,
                    )
                    nc.tensor.matmul(
                        poo[:, ts(cp, O)], hta[HP:P, ts(cp, P)],
                        w3st[HP:P, :], start=True, stop=True,
                    )
                nc.vector.tensor_tensor(
                    out_sb[:, :, 0, :],
                    poe[:].rearrange("q (j o) -> q j o", o=O),
                    b3rep[:].rearrange("q (j o) -> q j o", o=O),
                    mybir.AluOpType.add,
                )
                nc.vector.tensor_tensor(
                    out_sb[:, :, 1, :],
                    poo[:].rearrange("q (j o) -> q j o", o=O),
                    b3rep[:].rearrange("q (j o) -> q j o", o=O),
                    mybir.AluOpType.add,
                )
                nc.sync.dma_start(
                    out_d[p].rearrange("(q cp par) f -> q cp par f",
                                       q=P, cp=NPAIR_T, par=2),
                    out_sb[:],
                )

            return next_st, tail

        def body():
            st = prep(0)
            tail = None
            for p in range(PAIRS):
                st, tail = main(p, st, tail)
            tail()

        if reps == 1:
            body()
        else:
            with tc.For_i(0, reps, 1):
                body()


def build_program(reps=1):
    nc = bacc.Bacc("TRN2", target_bir_lowering=False, debug=False)
    x_d = nc.dram_tensor("x", [PAIRS, N, F], F32, kind="ExternalInput").ap()
    w_d = [
        nc.dram_tensor(f"w{k}", [H, F], F32, kind="ExternalInput").ap()
        for k in (1, 2, 3)
    ]
    b_d = [
        nc.dram_tensor(f"b{k}", [H], F32, kind="ExternalInput").ap()
        for k in (1, 2, 3)
    ]
    out_d = nc.dram_tensor("out", [PAIRS, N, O], F32, kind="ExternalOutput").ap()
    with tile.TileContext(nc) as tc:
        _emit(tc, x_d, w_d, b_d, out_d, reps=reps)
    nc.compile()
    return nc


def make_in_maps(x, W1, b1, W2, b2, W3, b3):
    xs = np.ascontiguousarray(np.asarray(x, np.float32).reshape(B * C, N, F))
    const = {
        "w1": np.ascontiguousarray(np.asarray(W1, np.float32)),
        "w2": np.ascontiguousarray(np.asarray(W2, np.float32)),
        "w3": np.ascontiguousarray(np.asarray(W3, np.float32)),
        "b1": np.ascontiguousarray(np.asarray(b1, np.float32)),
        "b2": np.ascontiguousarray(np.asarray(b2, np.float32)),
        "b3": np.ascontiguousarray(np.asarray(b3, np.float32)),
    }
    return [
        {"x": np.ascontiguousarray(xs[i * PAIRS : (i + 1) * PAIRS]), **const}
        for i in range(NCORES)
    ]


_NC_CACHE = {}


def kernel(x, W1, b1, W2, b2, W3, b3):
    from concourse.bass_utils import run_bass_kernel_spmd

    if "nc" not in _NC_CACHE:
        _NC_CACHE["nc"] = build_program()
    nc = _NC_CACHE["nc"]
    in_maps = make_in_maps(x, W1, b1, W2, b2, W3, b3)
    res = run_bass_kernel_spmd(nc, in_maps, list(range(NCORES))).results
    out = np.concatenate([res[i]["out"] for i in range(NCORES)], axis=0)
    return out.reshape(B, C, N, O)


# revision 15
# speedup vs baseline: 1.7275x; 1.0930x over previous
"""GCN message-passing kernel for 8 TRN2 NeuronCores.

Reference computation (per (b, c) pair, fp32 reference):
    e1  = x @ W1^T + b1          [N, H]
    e2  = x @ W2^T + b2          [N, H]
    adj = relu(e1 @ e2^T)        [N, N]
    h   = adj @ x                [N, F]
    out = h @ W3^T + b3          [N, O]

Sharding: the 32 (b, c) pairs are split 4-per-core across 8 cores;
weights are replicated. Each core runs an identical Bass program fully
fused in SBUF/PSUM (the N x N adjacency never touches HBM).

All matmul operands are bf16 (PSUM accumulation stays fp32); the 2e-2
rel-err gate leaves ample room (bf16 quantization contributes ~3e-3).

Parity layout: every K=64 contraction is packed two-per-PE via
tile_position row tiles (0,*) / (64,*), which the PE executes
concurrently (measured 2x). Row-block t of a pair lives on partition
half t%2: the [128,128] x-transposes emit (f@t-even ; f@t-odd) stacked,
and e1/e2/h^T inherit the layout, so adjacency, the e-matmuls and the
output projection all pair up. biases ride along on the mandatory
PSUM->SBUF copies (per-partition bias on ACT, tensor-tensor add on DVE)
instead of K=65 augmentation, keeping contractions pairable at K=64.

Layout trick: partition p of SBUF holds rows [16p, 16p+16) of the pair
(a pure row permutation, applied consistently to both sides of every
contraction and undone by the output DMA), which makes every HBM
transfer contiguous 4KB per partition.
"""

import sys

for _p in ("/opt/trn_rl_repo",):
    if _p not in sys.path:
        sys.path.insert(0, _p)

import numpy as np

import concourse.bass as bass
import concourse.tile as tile
from concourse import bacc, mybir
from concourse.bass import ts
from concourse.masks import make_identity

B, C, N, F = 4, 8, 2048, 64
H = 64
O = 64
NCORES = 8
PAIRS = (B * C) // NCORES  # 4 (b,c) pairs per core
P = 128                    # SBUF partitions
HP = 64                    # half-partition (PE row/col tile)
TBLK = N // P              # 16 row-blocks per pair
NPAIR_T = TBLK // 2        # 8 block-pairs (even t, odd t)
CH = 512                   # moving-operand chunk (one PSUM bank of fp32)
NH = N // 2                # 1024 columns per parity
NCH2 = NH // CH            # 2 chunks per parity
F32 = mybir.dt.float32
BF16 = mybir.dt.bfloat16

AF = mybir.ActivationFunctionType
ID = AF.Identity


class _Rotor:
    """Alternate PSUM->SBUF relu/copy work across DVE and ACT.
    (GPSIMD cannot access PSUM on TRN2, so Pool only gets SBUF-only work.)"""

    def __init__(self, nc):
        self.nc = nc
        self.i = 0

    def relu(self, out, in_):
        e = "da"[self.i % 2]
        self.i += 1
        if e == "d":
            self.nc.vector.tensor_scalar_max(out, in_, 0.0)
        else:
            self.nc.scalar.activation(out, in_, AF.Relu)

    def copy(self, out, in_):
        e = "da"[self.i % 2]
        self.i += 1
        if e == "d":
            self.nc.vector.tensor_copy(out, in_)
        else:
            self.nc.scalar.copy(out, in_)


def _emit(tc, x_d, w_d, b_d, out_d, reps=1):
    nc = tc.nc
    rot = _Rotor(nc)

    import contextlib

    with contextlib.ExitStack() as ctx:
        consts = ctx.enter_context(tc.tile_pool(name="consts", bufs=1))
        xpool = ctx.enter_context(tc.tile_pool(name="xp", bufs=2))
        xtpool = ctx.enter_context(tc.tile_pool(name="xt", bufs=2))
        epool = ctx.enter_context(tc.tile_pool(name="ep", bufs=2))
        adjpool = ctx.enter_context(tc.tile_pool(name="adj", bufs=12))
        htpool = ctx.enter_context(tc.tile_pool(name="ht", bufs=2))
        opool = ctx.enter_context(tc.tile_pool(name="op", bufs=2))
        ps_adj = ctx.enter_context(tc.tile_pool(name="psa", bufs=6, space="PSUM"))
        ps_h = ctx.enter_context(tc.tile_pool(name="psh", bufs=2, space="PSUM"))

        ident = consts.tile([P, P], F32)
        make_identity(nc, ident[:])
        identb = consts.tile([P, P], BF16)
        make_identity(nc, identb[:])

        # Weights W1/W2/W3 transposed to [f, h] bf16, duplicated on both
        # partition halves (for row-tile pairing); per-partition bias cols.
        wstats, bcols = [], []
        for k in range(3):
            wraw = consts.tile([H, F], F32, tag="wraw")
            nc.sync.dma_start(wraw[:], w_d[k][:])
            wst = consts.tile([P, H], BF16, tag=f"wst{k}")
            pw = ps_adj.tile([F, H], F32, tag="pa")
            nc.tensor.transpose(pw[:], wraw[:], ident[0:H, 0:H])
            nc.vector.tensor_copy(wst[0:HP, :], pw[:])
            nc.sync.dma_start(wst[HP:P, :], wst[0:HP, :])
            wstats.append(wst)
            if k < 2:
                bcol = consts.tile([P, 1], F32, tag=f"bc{k}")
                nc.sync.dma_start(bcol[0:HP, :], b_d[k].unsqueeze(1))
                nc.sync.dma_start(bcol[HP:P, :], b_d[k].unsqueeze(1))
                bcols.append(bcol)
        w1st, w2st, w3st = wstats
        b1c, b2c = bcols

        # b3 replicated [128, 512] (cols = 8 blocks x 64 o) for the fused
        # bias-add on the projection's PSUM->SBUF copy.
        b3row = consts.tile([1, O], F32)
        nc.sync.dma_start(b3row[:], b_d[2].unsqueeze(0))
        b3x8 = consts.tile([1, CH], F32)
        for j in range(CH // O):
            nc.vector.tensor_copy(b3x8[:, ts(j, O)], b3row[:])
        ones1 = consts.tile([1, P], F32)
        nc.gpsimd.memset(ones1[:], 1.0)
        b3rep = consts.tile([P, CH], F32)
        pb3 = ps_adj.tile([P, CH], F32, tag="pa")
        nc.tensor.matmul(pb3[:], ones1[:], b3x8[:], start=True, stop=True)
        nc.vector.tensor_copy(b3rep[:], pb3[:])

        def prep_load(p):
            """x load + bf16 round — emitted early in the previous pair's
            pipeline so the DMA/Pool latency is hidden long before the
            PE-side prep_compute needs x_bf."""
            x_sb = xpool.tile([P, TBLK, F], F32, tag="x_sb", name=f"x_sb{p}")
            nc.sync.dma_start(
                x_sb[:], x_d[p].rearrange("(q t) f -> q t f", q=P)
            )
            # SBUF->SBUF, so Pool can own it (frees DVE/ACT for PSUM work)
            x_bf = xpool.tile([P, TBLK, F], BF16, tag="x_bf", name=f"x_bf{p}")
            nc.gpsimd.tensor_copy(x_bf[:], x_sb[:])
            return x_bf

        def prep_compute(p, x_bf):
            """Parity transposes, e1/e2 (+ swapped-half copy of e1).

            xT in parity layout: column (c', q) holds f@t=2c' on rows
            0..63 and f@t=2c'+1 on rows 64..127."""
            xta = xtpool.tile([P, NH], BF16, tag="xta", name=f"xta{p}")
            for cp in range(NPAIR_T):
                pt = ps_adj.tile([P, P], BF16, tag="pa", name=f"pt{p}_{cp}")
                nc.tensor.transpose(
                    pt[:], x_bf[:, 2 * cp : 2 * cp + 2, :], identb[:]
                )
                rot.copy(xta[:, ts(cp, P)], pt[:])

            # e1/e2 in parity layout, paired diagonal tiles (0,0)/(64,64);
            # bias added on the ACT PSUM->SBUF copy (per-partition bias AP).
            e1t = epool.tile([P, NH], BF16, tag="e1t", name=f"e1t{p}")
            e2t = epool.tile([P, NH], BF16, tag="e2t", name=f"e2t{p}")
            for wst, bc, et in ((w1st, b1c, e1t), (w2st, b2c, e2t)):
                for g in range(NCH2):
                    pe2 = ps_adj.tile([P, CH], F32, tag="pa",
                                      name=f"pe{p}_{g}")
                    nc.tensor.matmul(
                        pe2[0:HP, :], wst[0:HP, :], xta[0:HP, ts(g, CH)],
                        start=True, stop=True,
                    )
                    nc.tensor.matmul(
                        pe2[HP:P, :], wst[HP:P, :], xta[HP:P, ts(g, CH)],
                        start=True, stop=True,
                    )
                    nc.scalar.activation(et[0:HP, ts(g, CH)], pe2[0:HP, :],
                                         ID, bias=bc[0:HP, :])
                    nc.scalar.activation(et[HP:P, ts(g, CH)], pe2[HP:P, :],
                                         ID, bias=bc[HP:P, :])
            # swapped-half copy of e1 so any (row-parity, col-parity) adj
            # instruction finds its moving operand on the right partitions.
            e1s = epool.tile([P, NH], BF16, tag="e1s", name=f"e1s{p}")
            nc.sync.dma_start(e1s[0:HP, :], e1t[HP:P, :])
            nc.sync.dma_start(e1s[HP:P, :], e1t[0:HP, :])
            return x_bf, e1t, e1s, e2t

        CHUNKS = [(pc, g) for g in range(NCH2) for pc in range(2)]
        SLOTS = [(u, pc, g) for u in range(NPAIR_T) for (pc, g) in CHUNKS]
        LAG = 4  # h-matmuls trail adjacency by one block-pair (4 slots)

        def main(p, st, tail_emit):
            """Flat software pipeline over 32 adjacency-chunk slots.
            Slot k emits: h for slot k-LAG, then the paired adjacency
            matmuls (row-tiles 0/64, concurrent on the PE) for slot k,
            then their relus (DVE / ACT). Every PE instruction's deps are
            >= LAG-1 slots old, so the PE queue never drains and pairing
            engages."""
            x_bf, e1t, e1s, e2t = st
            ph = [
                ps_h.tile([P, CH], F32, tag="ph", name=f"ph{p}_{g}")
                for g in range(NCH2)
            ]
            next_st = None
            next_xbf = None
            asbs = {}

            def emit_h(k):
                u, pc, g = SLOTS[k]
                for pt_ in range(2):
                    # skip_group_check: the interp's PSUM group tracker
                    # is partition-blind; the two half-bank groups are
                    # on disjoint partitions (HW zeroes per element).
                    nc.tensor.matmul(
                        ph[g][HP * pc : HP * pc + HP, :],
                        x_bf[:, 2 * u + pt_, :],
                        asbs.pop((u, pc, g, pt_))[:],
                        start=(u == 0 and pt_ == 0),
                        stop=(u == NPAIR_T - 1 and pt_ == 1),
                        skip_group_check=True,
                    )

            for k, (u, pc, g) in enumerate(SLOTS):
                if k >= LAG:
                    emit_h(k - LAG)
                pas = []
                for pt_ in range(2):  # t = 2u + pt_, row-tile 64*pt_
                    mv = e1t if pc == pt_ else e1s
                    pa = ps_adj.tile([P, CH], F32, tag="pa",
                                     name=f"pa{p}_{u}_{pc}_{g}_{pt_}")
                    nc.tensor.matmul(
                        pa[:],
                        e2t[HP * pt_ : HP * pt_ + HP, ts(u, P)],
                        mv[HP * pt_ : HP * pt_ + HP, ts(g, CH)],
                        start=True, stop=True,
                    )
                    pas.append(pa)
                for pt_ in range(2):
                    asb = adjpool.tile([P, CH], BF16, tag="asb",
                                       name=f"asb{p}_{u}_{pc}_{g}_{pt_}")
                    # deterministic per-slot engine split keeps DVE/ACT in
                    # lockstep so paired banks free together
                    if pt_ == 0:
                        nc.vector.tensor_scalar_max(asb[:], pas[pt_][:], 0.0)
                    else:
                        nc.scalar.activation(asb[:], pas[pt_][:], AF.Relu)
                    asbs[(u, pc, g, pt_)] = asb
                if k == 8 and tail_emit is not None:
                    tail_emit()
                    tail_emit = None
                if k == 2 and p + 1 < PAIRS:
                    next_xbf = prep_load(p + 1)
                if k == 20 and p + 1 < PAIRS:
                    next_st = prep_compute(p + 1, next_xbf)
            for k in range(len(SLOTS) - LAG, len(SLOTS)):
                emit_h(k)

            # hT -> SBUF (parity layout [128, NH]): frees the ph banks.
            hta = htpool.tile([P, NH], BF16, tag="hta", name=f"hta{p}")
            for g in range(NCH2):
                rot.copy(hta[0:HP, ts(g, CH)], ph[g][0:HP, :])
                rot.copy(hta[HP:P, ts(g, CH)], ph[g][HP:P, :])

            def tail():
                # out = h @ W3^T + b3: per t-block, stationary
                # hta[parity-half, c'-block] (paired row tiles), moving
                # W3^T; + b3 fused into the PSUM->SBUF tensor-tensor add.
                # out_sb free layout [cp, par, o] == [(t) o] row-major
                out_sb = opool.tile([P, NPAIR_T, 2, O], F32, tag="out_sb",
                                    name=f"out_sb{p}")
                poe = ps_adj.tile([P, CH], F32, tag="pa", name=f"poe{p}")
                poo = ps_adj.tile([P, CH], F32, tag="pa", name=f"poo{p}")
                for cp in range(NPAIR_T):
                    nc.tensor.matmul(
                        poe[:, ts(cp, O)], hta[0:HP, ts(cp, P)],
                        w3st[0:HP, :], start=True, stop=True,
                    )
                    nc.tensor.matmul(
                        poo[:, ts(cp, O)], hta[HP:P, ts(cp, P)],
                        w3st[HP:P, :], start=True, stop=True,
                    )
                nc.vector.tensor_tensor(
                    out_sb[:, :, 0, :],
                    poe[:].rearrange("q (j o) -> q j o", o=O),
                    b3rep[:].rearrange("q (j o) -> q j o", o=O),
                    mybir.AluOpType.add,
                )
                nc.vector.tensor_tensor(
                    out_sb[:, :, 1, :],
                    poo[:].rearrange("q (j o) -> q j o", o=O),
                    b3rep[:].rearrange("q (j o) -> q j o", o=O),
                    mybir.AluOpType.add,
                )
                nc.sync.dma_start(
                    out_d[p].rearrange("(q cp par) f -> q cp par f",
                                       q=P, cp=NPAIR_T, par=2),
                    out_sb[:],
                )

            return next_st, tail

        def body():
            st = prep_compute(0, prep_load(0))
            tail = None
            for p in range(PAIRS):
                st, tail = main(p, st, tail)
            tail()

        if reps == 1:
            body()
        else:
            with tc.For_i(0, reps, 1):
                body()


def build_program(reps=1):
    nc = bacc.Bacc("TRN2", target_bir_lowering=False, debug=False)
    x_d = nc.dram_tensor("x", [PAIRS, N, F], F32, kind="ExternalInput").ap()
    w_d = [
        nc.dram_tensor(f"w{k}", [H, F], F32, kind="ExternalInput").ap()
        for k in (1, 2, 3)
    ]
    b_d = [
        nc.dram_tensor(f"b{k}", [H], F32, kind="ExternalInput").ap()
        for k in (1, 2, 3)
    ]
    out_d = nc.dram_tensor("out", [PAIRS, N, O], F32, kind="ExternalOutput").ap()
    with tile.TileContext(nc) as tc:
        _emit(tc, x_d, w_d, b_d, out_d, reps=reps)
    nc.compile()
    return nc


def make_in_maps(x, W1, b1, W2, b2, W3, b3):
    xs = np.ascontiguousarray(np.asarray(x, np.float32).reshape(B * C, N, F))
    const = {
        "w1": np.ascontiguousarray(np.asarray(W1, np.float32)),
        "w2": np.ascontiguousarray(np.asarray(W2, np.float32)),
        "w3": np.ascontiguousarray(np.asarray(W3, np.float32)),
        "b1": np.ascontiguousarray(np.asarray(b1, np.float32)),
        "b2": np.ascontiguousarray(np.asarray(b2, np.float32)),
        "b3": np.ascontiguousarray(np.asarray(b3, np.float32)),
    }
    return [
        {"x": np.ascontiguousarray(xs[i * PAIRS : (i + 1) * PAIRS]), **const}
        for i in range(NCORES)
    ]


_NC_CACHE = {}


def kernel(x, W1, b1, W2, b2, W3, b3):
    from concourse.bass_utils import run_bass_kernel_spmd

    if "nc" not in _NC_CACHE:
        _NC_CACHE["nc"] = build_program()
    nc = _NC_CACHE["nc"]
    in_maps = make_in_maps(x, W1, b1, W2, b2, W3, b3)
    res = run_bass_kernel_spmd(nc, in_maps, list(range(NCORES))).results
    out = np.concatenate([res[i]["out"] for i in range(NCORES)], axis=0)
    return out.reshape(B, C, N, O)


# revision 21
# speedup vs baseline: 2.0983x; 1.2147x over previous
"""GCN message-passing kernel for 8 TRN2 NeuronCores.

Reference computation (per (b, c) pair, fp32 reference):
    e1  = x @ W1^T + b1          [N, H]
    e2  = x @ W2^T + b2          [N, H]
    adj = relu(e1 @ e2^T)        [N, N]
    h   = adj @ x                [N, F]
    out = h @ W3^T + b3          [N, O]

Sharding: the 32 (b, c) pairs are split 4-per-core across 8 cores;
weights are replicated. Each core runs an identical Bass program fully
fused in SBUF/PSUM (the N x N adjacency never touches HBM).

All matmul operands are bf16 (PSUM accumulation stays fp32); the 2e-2
rel-err gate leaves ample room (bf16 quantization contributes ~3e-3).

Parity layout: every K=64 contraction is packed two-per-PE via
tile_position row tiles (0,*) / (64,*), which the PE executes
concurrently (measured 2x). Row-block t of a pair lives on partition
half t%2: the [128,128] x-transposes emit (f@t-even ; f@t-odd) stacked,
and e1/e2/h^T inherit the layout, so adjacency, the e-matmuls and the
output projection all pair up. biases ride along on the mandatory
PSUM->SBUF copies (per-partition bias on ACT, tensor-tensor add on DVE)
instead of K=65 augmentation, keeping contractions pairable at K=64.

Layout trick: partition p of SBUF holds rows [16p, 16p+16) of the pair
(a pure row permutation, applied consistently to both sides of every
contraction and undone by the output DMA), which makes every HBM
transfer contiguous 4KB per partition.
"""

import sys

for _p in ("/opt/trn_rl_repo",):
    if _p not in sys.path:
        sys.path.insert(0, _p)

import numpy as np

import concourse.bass as bass
import concourse.tile as tile
from concourse import bacc, mybir
from concourse.bass import ts
from concourse.masks import make_identity

B, C, N, F = 4, 8, 2048, 64
H = 64
O = 64
NCORES = 8
PAIRS = (B * C) // NCORES  # 4 (b,c) pairs per core
P = 128                    # SBUF partitions
HP = 64                    # half-partition (PE row/col tile)
TBLK = N // P              # 16 row-blocks per pair
NPAIR_T = TBLK // 2        # 8 block-pairs (even t, odd t)
CH = 512                   # moving-operand chunk (one PSUM bank of fp32)
NH = N // 2                # 1024 columns per parity
NCH2 = NH // CH            # 2 chunks per parity
F32 = mybir.dt.float32
BF16 = mybir.dt.bfloat16

AF = mybir.ActivationFunctionType
ID = AF.Identity


class _Rotor:
    """Alternate PSUM->SBUF relu/copy work across DVE and ACT.
    (GPSIMD cannot access PSUM on TRN2, so Pool only gets SBUF-only work.)"""

    def __init__(self, nc):
        self.nc = nc
        self.i = 0

    def relu(self, out, in_):
        e = "da"[self.i % 2]
        self.i += 1
        if e == "d":
            self.nc.vector.tensor_scalar_max(out, in_, 0.0)
        else:
            self.nc.scalar.activation(out, in_, AF.Relu)

    def copy(self, out, in_):
        e = "da"[self.i % 2]
        self.i += 1
        if e == "d":
            self.nc.vector.tensor_copy(out, in_)
        else:
            self.nc.scalar.copy(out, in_)


def _emit(tc, x_d, w_d, b_d, out_d, reps=1, variant="full"):
    nc = tc.nc
    rot = _Rotor(nc)

    import contextlib

    with contextlib.ExitStack() as ctx:
        consts = ctx.enter_context(tc.tile_pool(name="consts", bufs=1))
        xpool = ctx.enter_context(tc.tile_pool(name="xp", bufs=2))
        xtpool = ctx.enter_context(tc.tile_pool(name="xt", bufs=2))
        epool = ctx.enter_context(tc.tile_pool(name="ep", bufs=2))
        adjpool = ctx.enter_context(tc.tile_pool(name="adj", bufs=12))
        htpool = ctx.enter_context(tc.tile_pool(name="ht", bufs=2))
        opool = ctx.enter_context(tc.tile_pool(name="op", bufs=2))
        ps_adj = ctx.enter_context(tc.tile_pool(name="psa", bufs=6, space="PSUM"))
        ps_h = ctx.enter_context(tc.tile_pool(name="psh", bufs=2, space="PSUM"))

        ident = consts.tile([P, P], F32)
        make_identity(nc, ident[:])
        identb = consts.tile([P, P], BF16)
        make_identity(nc, identb[:])

        # Weights W1/W2/W3 transposed to [f, h] bf16, duplicated on both
        # partition halves (for row-tile pairing); per-partition bias cols.
        wstats, bcols = [], []
        for k in range(3):
            wraw = consts.tile([H, F], F32, tag="wraw")
            nc.sync.dma_start(wraw[:], w_d[k][:])
            wst = consts.tile([P, H], BF16, tag=f"wst{k}")
            pw = ps_adj.tile([F, H], F32, tag="pa")
            nc.tensor.transpose(pw[:], wraw[:], ident[0:H, 0:H])
            nc.vector.tensor_copy(wst[0:HP, :], pw[:])
            nc.sync.dma_start(wst[HP:P, :], wst[0:HP, :])
            wstats.append(wst)
            if k < 2:
                bcol = consts.tile([P, 1], F32, tag=f"bc{k}")
                nc.sync.dma_start(bcol[0:HP, :], b_d[k].unsqueeze(1))
                nc.sync.dma_start(bcol[HP:P, :], b_d[k].unsqueeze(1))
                bcols.append(bcol)
        w1st, w2st, w3st = wstats
        b1c, b2c = bcols

        # b3 replicated [128, 512] (cols = 8 blocks x 64 o) for the fused
        # bias-add on the projection's PSUM->SBUF copy.
        b3row = consts.tile([1, O], F32)
        nc.sync.dma_start(b3row[:], b_d[2].unsqueeze(0))
        b3x8 = consts.tile([1, CH], F32)
        for j in range(CH // O):
            nc.vector.tensor_copy(b3x8[:, ts(j, O)], b3row[:])
        ones1 = consts.tile([1, P], F32)
        nc.gpsimd.memset(ones1[:], 1.0)
        b3rep = consts.tile([P, CH], F32)
        pb3 = ps_adj.tile([P, CH], F32, tag="pa")
        nc.tensor.matmul(pb3[:], ones1[:], b3x8[:], start=True, stop=True)
        nc.vector.tensor_copy(b3rep[:], pb3[:])

        # timing-variant scratch: constant asb stand-in (pe_only)
        asb_const = None
        if variant == "pe_only":
            asb_const = consts.tile([P, CH], BF16)
            nc.gpsimd.memset(asb_const[:], 0.5)

        def prep_load(p):
            """x load + bf16 round — emitted early in the previous pair's
            pipeline so the DMA/Pool latency is hidden long before the
            PE-side prep_compute needs x_bf."""
            x_sb = xpool.tile([P, TBLK, F], F32, tag="x_sb", name=f"x_sb{p}")
            nc.sync.dma_start(
                x_sb[:], x_d[p].rearrange("(q t) f -> q t f", q=P)
            )
            # SBUF->SBUF, so Pool can own it (frees DVE/ACT for PSUM work)
            x_bf = xpool.tile([P, TBLK, F], BF16, tag="x_bf", name=f"x_bf{p}")
            nc.gpsimd.tensor_copy(x_bf[:], x_sb[:])
            return x_bf

        def prep_compute(p, x_bf):
            """Parity transposes, e1/e2 (+ swapped-half copy of e1).

            xT in parity layout: column (c', q) holds f@t=2c' on rows
            0..63 and f@t=2c'+1 on rows 64..127."""
            xta = xtpool.tile([P, NH], BF16, tag="xta", name=f"xta{p}")
            for cp in range(NPAIR_T):
                pt = ps_adj.tile([P, P], BF16, tag="pa", name=f"pt{p}_{cp}")
                nc.tensor.transpose(
                    pt[:], x_bf[:, 2 * cp : 2 * cp + 2, :], identb[:]
                )
                rot.copy(xta[:, ts(cp, P)], pt[:])

            # e1/e2 in parity layout, paired diagonal tiles (0,0)/(64,64);
            # bias added on the ACT PSUM->SBUF copy (per-partition bias AP).
            e1t = epool.tile([P, NH], BF16, tag="e1t", name=f"e1t{p}")
            e2t = epool.tile([P, NH], BF16, tag="e2t", name=f"e2t{p}")
            for wst, bc, et in ((w1st, b1c, e1t), (w2st, b2c, e2t)):
                for g in range(NCH2):
                    pe2 = ps_adj.tile([P, CH], F32, tag="pa",
                                      name=f"pe{p}_{g}")
                    nc.tensor.matmul(
                        pe2[0:HP, :], wst[0:HP, :], xta[0:HP, ts(g, CH)],
                        start=True, stop=True,
                    )
                    nc.tensor.matmul(
                        pe2[HP:P, :], wst[HP:P, :], xta[HP:P, ts(g, CH)],
                        start=True, stop=True,
                    )
                    nc.scalar.activation(et[0:HP, ts(g, CH)], pe2[0:HP, :],
                                         ID, bias=bc[0:HP, :])
                    nc.scalar.activation(et[HP:P, ts(g, CH)], pe2[HP:P, :],
                                         ID, bias=bc[HP:P, :])
            # swapped-half copy of e1 so any (row-parity, col-parity) adj
            # instruction finds its moving operand on the right partitions.
            e1s = epool.tile([P, NH], BF16, tag="e1s", name=f"e1s{p}")
            nc.sync.dma_start(e1s[0:HP, :], e1t[HP:P, :])
            nc.sync.dma_start(e1s[HP:P, :], e1t[0:HP, :])
            return x_bf, e1t, e1s, e2t

        CHUNKS = [(pc, g) for g in range(NCH2) for pc in range(2)]
        SLOTS = [(u, pc, g) for u in range(NPAIR_T) for (pc, g) in CHUNKS]
        LAG = 4  # h-matmuls trail adjacency by one block-pair (4 slots)

        def main(p, st, tail_emit):
            """Flat software pipeline over 32 adjacency-chunk slots.
            Slot k emits: h for slot k-LAG, then the paired adjacency
            matmuls (row-tiles 0/64, concurrent on the PE) for slot k,
            then their relus (DVE / ACT). Every PE instruction's deps are
            >= LAG-1 slots old, so the PE queue never drains and pairing
            engages."""
            x_bf, e1t, e1s, e2t = st
            ph = [
                ps_h.tile([P, CH], F32, tag="ph", name=f"ph{p}_{g}")
                for g in range(NCH2)
            ]
            next_st = None
            next_xbf = None
            asbs = {}

            def emit_h(k):
                u, pc, g = SLOTS[k]
                for pt_ in range(2):
                    src = (asb_const if variant == "pe_only"
                           else asbs.pop((u, pc, g, pt_)))
                    # skip_group_check: the interp's PSUM group tracker
                    # is partition-blind; the two half-bank groups are
                    # on disjoint partitions (HW zeroes per element).
                    nc.tensor.matmul(
                        ph[g][HP * pc : HP * pc + HP, :],
                        x_bf[:, 2 * u + pt_, :],
                        src[:],
                        start=(u == 0 and pt_ == 0),
                        stop=(u == NPAIR_T - 1 and pt_ == 1),
                        skip_group_check=True,
                    )

            for k, (u, pc, g) in enumerate(SLOTS):
                if k >= LAG:
                    emit_h(k - LAG)
                pas = []
                for pt_ in range(2):  # t = 2u + pt_, row-tile 64*pt_
                    mv = e1t if pc == pt_ else e1s
                    pa = ps_adj.tile([P, CH], F32, tag="pa",
                                     name=f"pa{p}_{u}_{pc}_{g}_{pt_}")
                    nc.tensor.matmul(
                        pa[:],
                        e2t[HP * pt_ : HP * pt_ + HP, ts(u, P)],
                        mv[HP * pt_ : HP * pt_ + HP, ts(g, CH)],
                        start=True, stop=True,
                    )
                    pas.append(pa)
                for pt_ in range(2):
                    if variant == "pe_only":
                        continue
                    asb = adjpool.tile([P, CH], BF16, tag="asb",
                                       name=f"asb{p}_{u}_{pc}_{g}_{pt_}")
                    # deterministic per-slot engine split keeps DVE/ACT in
                    # lockstep so paired banks free together
                    if pt_ == 0:
                        nc.vector.tensor_scalar_max(asb[:], pas[pt_][:], 0.0)
                    else:
                        nc.scalar.activation(asb[:], pas[pt_][:], AF.Relu)
                    asbs[(u, pc, g, pt_)] = asb
                if k == 8 and tail_emit is not None:
                    tail_emit()
                    tail_emit = None
                if k == 2 and p + 1 < PAIRS:
                    next_xbf = prep_load(p + 1)
                if k == 20 and p + 1 < PAIRS:
                    next_st = prep_compute(p + 1, next_xbf)
            for k in range(len(SLOTS) - LAG, len(SLOTS)):
                emit_h(k)

            # hT -> SBUF (parity layout [128, NH]): frees the ph banks.
            hta = htpool.tile([P, NH], BF16, tag="hta", name=f"hta{p}")
            for g in range(NCH2):
                rot.copy(hta[0:HP, ts(g, CH)], ph[g][0:HP, :])
                rot.copy(hta[HP:P, ts(g, CH)], ph[g][HP:P, :])

            def tail():
                # out = h @ W3^T + b3: per t-block, stationary
                # hta[parity-half, c'-block] (paired row tiles), moving
                # W3^T; + b3 fused into the PSUM->SBUF tensor-tensor add.
                # out_sb free layout [cp, par, o] == [(t) o] row-major
                out_sb = opool.tile([P, NPAIR_T, 2, O], F32, tag="out_sb",
                                    name=f"out_sb{p}")
                poe = ps_adj.tile([P, CH], F32, tag="pa", name=f"poe{p}")
                poo = ps_adj.tile([P, CH], F32, tag="pa", name=f"poo{p}")
                for cp in range(NPAIR_T):
                    nc.tensor.matmul(
                        poe[:, ts(cp, O)], hta[0:HP, ts(cp, P)],
                        w3st[0:HP, :], start=True, stop=True,
                    )
                    nc.tensor.matmul(
                        poo[:, ts(cp, O)], hta[HP:P, ts(cp, P)],
                        w3st[HP:P, :], start=True, stop=True,
                    )
                nc.vector.tensor_tensor(
                    out_sb[:, :, 0, :],
                    poe[:].rearrange("q (j o) -> q j o", o=O),
                    b3rep[:].rearrange("q (j o) -> q j o", o=O),
                    mybir.AluOpType.add,
                )
                nc.vector.tensor_tensor(
                    out_sb[:, :, 1, :],
                    poo[:].rearrange("q (j o) -> q j o", o=O),
                    b3rep[:].rearrange("q (j o) -> q j o", o=O),
                    mybir.AluOpType.add,
                )
                nc.sync.dma_start(
                    out_d[p].rearrange("(q cp par) f -> q cp par f",
                                       q=P, cp=NPAIR_T, par=2),
                    out_sb[:],
                )

            return next_st, tail

        def body():
            st = prep_compute(0, prep_load(0))
            tail = None
            for p in range(PAIRS):
                st, tail = main(p, st, tail)
            tail()

        if reps == 1:
            body()
        else:
            with tc.For_i(0, reps, 1):
                body()


def build_program(reps=1, variant=None):
    import os
    if variant is None:
        variant = os.environ.get("KVAR", "full")
    nc = bacc.Bacc("TRN2", target_bir_lowering=False, debug=False)
    x_d = nc.dram_tensor("x", [PAIRS, N, F], F32, kind="ExternalInput").ap()
    w_d = [
        nc.dram_tensor(f"w{k}", [H, F], F32, kind="ExternalInput").ap()
        for k in (1, 2, 3)
    ]
    b_d = [
        nc.dram_tensor(f"b{k}", [H], F32, kind="ExternalInput").ap()
        for k in (1, 2, 3)
    ]
    out_d = nc.dram_tensor("out", [PAIRS, N, O], F32, kind="ExternalOutput").ap()
    with tile.TileContext(nc) as tc:
        _emit(tc, x_d, w_d, b_d, out_d, reps=reps, variant=variant)
    nc.compile()
    return nc


def make_in_maps(x, W1, b1, W2, b2, W3, b3):
    xs = np.ascontiguousarray(np.asarray(x, np.float32).reshape(B * C, N, F))
    const = {
        "w1": np.ascontiguousarray(np.asarray(W1, np.float32)),
        "w2": np.ascontiguousarray(np.asarray(W2, np.float32)),
        "w3": np.ascontiguousarray(np.asarray(W3, np.float32)),
        "b1": np.ascontiguousarray(np.asarray(b1, np.float32)),
        "b2": np.ascontiguousarray(np.asarray(b2, np.float32)),
        "b3": np.ascontiguousarray(np.asarray(b3, np.float32)),
    }
    return [
        {"x": np.ascontiguousarray(xs[i * PAIRS : (i + 1) * PAIRS]), **const}
        for i in range(NCORES)
    ]


_NC_CACHE = {}


def kernel(x, W1, b1, W2, b2, W3, b3):
    from concourse.bass_utils import run_bass_kernel_spmd

    if "nc" not in _NC_CACHE:
        _NC_CACHE["nc"] = build_program()
    nc = _NC_CACHE["nc"]
    in_maps = make_in_maps(x, W1, b1, W2, b2, W3, b3)
    res = run_bass_kernel_spmd(nc, in_maps, list(range(NCORES))).results
    out = np.concatenate([res[i]["out"] for i in range(NCORES)], axis=0)
    return out.reshape(B, C, N, O)
